# revision 12
# baseline (speedup 1.0000x reference)
"""HSIC pairwise loss kernel for trn2 (8 NeuronCores), fp8 DoubleRow version.

Math: reference builds K_c = (w^2 w^2T) * (E_c E_c^T), M_c = R K_c, and sums
tr(M_i M_j) over i<j. With F_c = w^2 * E_c (row scaling), R the centering
matrix (idempotent), A_ij = F_i^T F_j, s_c = F_c^T 1:
    loss = sum_{i<j} ||A_ij - s_i s_j^T / n||_F^2 / (n-1)^2.

Device work: the 45 cross-chunk Gram blocks A_ij ([256,256], contraction
over n=4096) at half-chunk (128-col "unit") granularity. Host prescales
F = w^2 * X (exact *16 power-of-2 gain), quantizes to fp8-e4m3 (TRN
variant, max 240), and computes the column sums s in float64 for free.

Each core is one K_{2,3}: parents {a,b} x {c,d,e} -> 6 chunk-pair blocks.
All 8 cores run the same program: 10 units (1280 cols) laid out as
[c0 c1 d0 d1 | a0 a1 b0 b1 | e0 e1], 6 matmul windows of 512 output cols
(stat slots 4-7 x mov slots 0-3; stat slots 8-9 x mov slots 4-7), fp8
DoubleRow (256-row contraction per instruction, 2 MAC/cell/cycle), 16
double-k-tiles accumulated in 6 PSUM banks. The 8 K23s cover all 45
chunk pairs (3 duplicates, deduped on host). PSUM is copied out as bf16;
host applies the rank-1 centering correction and final reduce in fp64.
"""

import numpy as np
from contextlib import ExitStack

import concourse.bass as bass
import concourse.tile as tile
from concourse import bacc, mybir
from concourse import bass_utils

N = 4096
NUM_CHUNKS = 10
DK = 16                  # double-k-tiles of 256 rows
UNITS = 10               # units (128-col half-chunks) per core
C = UNITS * 128          # 1280 data cols per core
SC = 16.0                # exact power-of-2 host prescale (avoids fp8 subnormals)
OUT_COLS = 6 * 512       # 3072

# 8 K_{2,3} parts (L=(a,b), R=(c,d,e)) covering all 45 chunk pairs.
PARTS = [
    ((3, 7), (1, 4, 5)),
    ((0, 8), (3, 5, 7)),
    ((2, 9), (3, 5, 8)),
    ((0, 5), (1, 4, 6)),
    ((4, 6), (1, 2, 8)),
    ((0, 1), (2, 8, 9)),
    ((7, 9), (2, 4, 6)),
    ((6, 7), (3, 4, 9)),
]

# (stat_slot, mov_start_col, n_cols): identical for every core.
WINDOWS = [
    (4, 0, 512),
    (5, 0, 512),
    (6, 0, 512),
    (7, 0, 512),
    (8, 512, 512),
    (9, 512, 512),
]


def _part_units(L, R):
    a, b = L
    c, d, e = R
    parents = [c, d, a, b, e]
    units = []
    for P in parents:
        units += [2 * P, 2 * P + 1]
    return units


def _check_cover():
    need = {(i, j) for i in range(NUM_CHUNKS) for j in range(i + 1, NUM_CHUNKS)}
    got = set()
    for L, R in PARTS:
        units = _part_units(L, R)
        for (s, mc, nw) in WINDOWS:
            su = units[s]
            for t in range(nw // 128):
                mu = units[mc // 128 + t]
                pi, pj = su // 2, mu // 2
                assert pi != pj
                got.add((min(pi, pj), max(pi, pj)))
    assert need <= got, need - got


_check_cover()

_CACHE = {}


def _build():
    f8 = mybir.dt.float8e4
    f32 = mybir.dt.float32
    bf16 = mybir.dt.bfloat16
    nc = bacc.Bacc("TRN2", target_bir_lowering=False, debug=False,
                   num_devices=8, enable_partition_id=False)
    # First pair split into 2 half-dktile transfers (smaller first
    # dependency -> earlier first matmul); rest as dktile pairs with
    # 5120B partition lines for DMA packet efficiency.
    x0 = nc.dram_tensor("x0", [2, 128, 2, C], f8, kind="ExternalInput").ap()
    x = nc.dram_tensor("x", [DK // 2 - 1, 128, 4, C], f8,
                       kind="ExternalInput").ap()
    out = nc.dram_tensor("out", [128, OUT_COLS], bf16,
                         kind="ExternalOutput").ap()

    with tile.TileContext(nc) as tc:
        with ExitStack() as ctx:
            wpool = ctx.enter_context(tc.tile_pool(name="wu", bufs=1))
            xpool = ctx.enter_context(tc.tile_pool(name="xs", bufs=4))
            psum = ctx.enter_context(tc.tile_pool(name="ps", bufs=1,
                                                  space="PSUM"))
            opool = ctx.enter_context(tc.tile_pool(name="o", bufs=1))

            ps = []
            for i in range(len(WINDOWS)):
                ps.append(psum.tile([128, 512], f32, tag=f"ps{i}",
                                    name=f"ps{i}"))

            # HAM warmup: junk matmuls keep the PE busy from engine boot
            # until the first data tile lands, so real matmuls run at
            # 2.4 GHz (K=8/8) from the start. Results land in a scratch
            # PSUM bank that is never read.
            wt = wpool.tile([128, 2, 256], f8, tag="warm")
            nc.vector.memset(wt[:], 0.0)
            wps = psum.tile([128, 512], f32, tag="wps", name="wps")
            for _ in range(14):
                nc.tensor.matmul(
                    wps[:, 0:256], wt[:, :, 0:128], wt[:, :, 0:256],
                    start=True, stop=True,
                    perf_mode=mybir.MatmulPerfMode.DoubleRow,
                )

            def halfmm(xh, first, last):
                for wi, (s, mc, nw) in enumerate(WINDOWS):
                    nc.tensor.matmul(
                        ps[wi][:, 0:nw],
                        xh[:, :, s * 128:(s + 1) * 128],
                        xh[:, :, mc:mc + nw],
                        start=first, stop=last,
                        perf_mode=mybir.MatmulPerfMode.DoubleRow,
                    )

            for h in range(2):
                xt0 = xpool.tile([128, 2, C], f8)
                nc.sync.dma_start(xt0[:], x0[h])
                halfmm(xt0, h == 0, False)

            for kk in range(DK // 2 - 1):
                xt = xpool.tile([128, 4, C], f8)
                nc.sync.dma_start(xt[:], x[kk])
                for half in range(2):
                    halfmm(xt[:, 2 * half:2 * half + 2, :], False,
                           kk == DK // 2 - 2 and half == 1)

            # copies (vector+scalar in parallel) with per-window output DMA
            ot = opool.tile([128, OUT_COLS], bf16)
            for wi in range(len(WINDOWS)):
                sl = ot[:, wi * 512:(wi + 1) * 512]
                if wi % 2 == 0:
                    nc.vector.tensor_copy(sl, ps[wi][:])
                else:
                    nc.scalar.copy(sl, ps[wi][:])
                nc.sync.dma_start(out[:, wi * 512:(wi + 1) * 512], sl)
    nc.compile()
    return nc


def _get_nc():
    if "nc" not in _CACHE:
        _CACHE["nc"] = _build()
    return _CACHE["nc"]


def _in_maps(F8):
    np8 = mybir.dt.np(mybir.dt.float8e4)
    maps = []
    for L, R in PARTS:
        units = _part_units(L, R)
        cols = np.concatenate(
            [np.arange(u * 128, (u + 1) * 128) for u in units])
        Xc = F8[:, cols]                                   # [4096, 1280]
        X0 = Xc[:512].reshape(2, 2, 128, C).transpose(0, 2, 1, 3)
        Xr = Xc[512:].reshape(DK // 2 - 1, 4, 128, C).transpose(0, 2, 1, 3)
        maps.append({
            "x0": np.ascontiguousarray(X0).astype(np8, copy=False),
            "x": np.ascontiguousarray(Xr).astype(np8, copy=False),
        })
    return maps


def _assemble(outs, s):
    inv = 1.0 / (SC * SC)
    quad = {}
    for core, (L, R) in enumerate(PARTS):
        o = outs[core].astype(np.float64) * inv
        units = _part_units(L, R)
        for wi, (ss, mc, nw) in enumerate(WINDOWS):
            su = units[ss]
            for t in range(nw // 128):
                mu = units[mc // 128 + t]
                key = (su, mu)
                if key not in quad:
                    quad[key] = o[:, wi * 512 + t * 128:wi * 512 + (t + 1) * 128]
    loss = 0.0
    for i in range(NUM_CHUNKS):
        s_i = s[i * 256:(i + 1) * 256]
        for j in range(i + 1, NUM_CHUNKS):
            s_j = s[j * 256:(j + 1) * 256]
            A = np.empty((256, 256))
            for a in range(2):
                for b in range(2):
                    u, v = 2 * i + a, 2 * j + b
                    q = quad[(u, v)] if (u, v) in quad else quad[(v, u)].T
                    A[a * 128:(a + 1) * 128, b * 128:(b + 1) * 128] = q
            Cm = A - np.outer(s_i, s_j) / float(N)
            loss += float((Cm * Cm).sum())
    loss /= float((N - 1) * (N - 1))
    return np.asarray([loss], np.float32)


def kernel(final_readout, weight, _trace=False):
    X = np.asarray(final_readout, np.float32)
    w = np.asarray(weight, np.float32)
    F = (w.astype(np.float64) ** 2) * X.astype(np.float64)
    s = F.sum(axis=0)                       # exact column sums, fp64
    np8 = mybir.dt.np(mybir.dt.float8e4)
    F8 = np.clip(F * SC, -240.0, 240.0).astype(np8)
    nc = _get_nc()
    res = bass_utils.run_bass_kernel_spmd(
        nc, _in_maps(F8), core_ids=list(range(8)), trace=_trace)
    _CACHE["last_results"] = res
    return _assemble([r["out"] for r in res.results], s)
